# revision 1
# baseline (speedup 1.0000x reference)
"""LocallyConnected2d kernel for 8 Trainium2 NeuronCores.

Problem:  out[b,o,h,w] = sum_{c,k} x[b, c, h+ki, w+kj] * W[o, c, h, w, k] + bias[o,h,w]
  B=8, C=16, H=W=64, O=32, K=3, HO=WO=62.

Sharding: each core owns 8 output rows (H_out axis), full O and batch.
  Weight slice (~9.1 MB/core) dominates HBM traffic -> memory roofline.

Compute mapping (per core): for each output location (h,w) the 144-term
reduction over (c, k) runs on the tensor engine as two accumulating fp32
matmuls; 4 locations (4 consecutive h rows, same w) run concurrently in the
four 32-column groups of the PE array via tile_position:
  chunk1: contraction (k<8, c) = 128 partitions, per-location weights
          stationary [128, o=32], patches moving [128, b=8].
  chunk2: contraction c = 16 partitions (k=8). Column group j reads its
          patches from xrep's k=2j block at a compensating offset
          (shift identity: x[h+2,w+2] = block(s)[h+dy, w+dx], (dy,dx)=(2,2)-s)
          so no separate narrow x tensor is needed; its weights sit in row
          block 32j of a dense 128-partition tensor.
  64 rounds accumulate into one full PSUM bank (opened/closed by K=1
  zero-weight matmuls); one DVE tensor_tensor per bank adds bias (broadcast
  AP along b) while evacuating.

The im2col tensor xrep[(k,c), (b, y<10, x<64)] is built on the HOST so it
lands in one contiguous full-width DMA.  Weight stream rides the SP HWDGE
ring; everything else rides the ACT ring so the transfers overlap.
"""

import numpy as np

import concourse.bacc as bacc
import concourse.mybir as mybir
from concourse import tile
from concourse.bass_utils import run_bass_kernel_spmd

F32 = mybir.dt.float32
BF16 = mybir.dt.bfloat16
USE_BF16 = False          # compute/weight dtype for w1/w2/xrep

N_CORES = 8
B, C, H, W = 8, 16, 64, 64
O, K = 32, 3
HO = WO = 62
ROWS = 8                # output rows per core
YIN = ROWS + K - 1      # input rows needed per core (10)
G = 2 * WO              # rounds per core (hb in {0,1} x w in [0,62)) = 124
ROW_STARTS = [0, 8, 16, 24, 32, 40, 48, 54]
W1_SIZES = [17, 17, 17, 17, 17, 12, 16, 11]       # rounds per chunk
W1_STARTS = list(np.cumsum([0] + W1_SIZES[:-1]))
XREPW = 64              # padded x width in xrep (62+2 used)
BANK_SIZES = [31, 31, 31, 31]       # rounds per PSUM bank
BANK_STARTS = list(np.cumsum([0] + BANK_SIZES[:-1]))
RPB = 31                # w2 transfer quarter (bank-independent)
# chunk2 offset (dy, dx) = (2,2) - shift(k=2j) per column group j
C2_DELTA = [(2, 2), (2, 0), (1, 1), (0, 2)]

TRACE = False
TRACE_KWARGS = {}
LAST_RESULT = None
_PROGRAM = None
_PROGRAM_KEY = None


def build_program():
    DT = BF16 if USE_BF16 else F32
    nc = bacc.Bacc(
        "TRN2", target_bir_lowering=False, debug=False, num_devices=N_CORES
    )
    w1_d = nc.dram_tensor("w1", [128, G * 128], DT, kind="ExternalInput")
    w2_d = nc.dram_tensor("w2", [128, G * 32], DT, kind="ExternalInput")
    xr_d = nc.dram_tensor("xr", [128, B * YIN * XREPW], DT,
                          kind="ExternalInput")
    bv_d = nc.dram_tensor("bv", [128, G], F32, kind="ExternalInput")
    out_d = nc.dram_tensor("out", [128, G * B], F32, kind="ExternalOutput")

    with tile.TileContext(nc) as tc:
        with tc.tile_pool(name="const", bufs=1) as cpool, \
             tc.tile_pool(name="w1p", bufs=8) as w1p, \
             tc.tile_pool(name="psum", bufs=1, space="PSUM") as ppool:
            # xrep free layout is (x, y, b): x-range splits are
            # contiguous, so the stream lands in 4 pieces and the PE can
            # start after the first ~320 KB; rhs reads are 8 contiguous
            # b elements.
            XP = XREPW // 2                       # x per piece (32)
            PCOLS = XP * YIN * B                  # cols per piece (1280)
            xps = [cpool.tile([128, PCOLS], DT, tag=f"xp{i}",
                              name=f"xp{i}")
                   for i in range(2)]
            w2_sb = cpool.tile([128, G * 32], DT)

            def load_xp(i):
                nc.scalar.dma_start(
                    xps[i][:], xr_d[:, i * PCOLS:(i + 1) * PCOLS])

            def load_w2(part):                    # bank-aligned quarters
                c0 = part * RPB * 32
                sz = min(RPB, G - part * RPB) * 32
                nc.scalar.dma_start(
                    w2_sb[:, c0:c0 + sz], w2_d[:, c0:c0 + sz])

            # order: everything round 0 touches first, then ascending w
            load_xp(0)
            load_w2(0)
            load_w2(1)
            load_xp(1)
            load_w2(2)
            load_w2(3)
            bias_sb = cpool.tile([128, G], F32)
            nc.scalar.dma_start(bias_sb[:], bv_d[:])
            out_sb = cpool.tile([128, G * B], F32)
            # bank open/close dummies use bf16 zeros: 1 cycle/row on the PE
            # instead of fp32's 4, and the values written are exact zeros
            zb = cpool.tile([1, 640], BF16)
            nc.vector.memset(zb[:], 0.0)

            def rhs_ap(p0, np_, y, x):
                i, xi = divmod(x, XP)
                c0 = (xi * YIN + y) * B
                return xps[i][p0:p0 + np_, c0:c0 + B]

            # weight chunks loaded lazily so each chunk's DMA overlaps the
            # previous chunk's matmuls
            w1_tiles = {}

            def get_w1(t):
                if t not in w1_tiles:
                    w1_sb = w1p.tile([128, max(W1_SIZES) * 128], DT,
                                     tag="w1")
                    c0, sz = W1_STARTS[t] * 128, W1_SIZES[t] * 128
                    # the last chunk rides the ACT ring so the two
                    # HWDGE queues finish together
                    eng = nc.scalar if t >= 6 else nc.sync
                    eng.dma_start(
                        w1_sb[:, :sz], w1_d[:, c0:c0 + sz]
                    )
                    w1_tiles[t] = w1_sb
                return w1_tiles[t]

            def w1_chunk_of(g):
                for t in range(len(W1_SIZES)):
                    if g < W1_STARTS[t] + W1_SIZES[t]:
                        return t, g - W1_STARTS[t]
                raise AssertionError(g)

            for bank in range(len(BANK_SIZES)):
                g0 = BANK_STARTS[bank]
                nb = BANK_SIZES[bank]
                ps = ppool.tile([128, 512], F32, tag=f"ps{bank}")
                # open the bank's accumulation group across all 128
                # partitions and clear the whole 2 KiB zero region
                nc.tensor.matmul(
                    ps[:, :],
                    lhsT=zb[:, 0:128],
                    rhs=zb[:, 128:640],
                    start=True, stop=False,
                    tile_position=(0, 0),
                )
                for r in range(nb):
                    g = g0 + r
                    hb, w = divmod(g, WO)
                    t, gl = w1_chunk_of(g)
                    w1_sb = get_w1(t)
                    for j in range(4):
                        h = hb * 4 + j
                        dy, dx = C2_DELTA[j]
                        nc.tensor.matmul(
                            ps[32 * j:32 * (j + 1), r * B:(r + 1) * B],
                            lhsT=w1_sb[:, gl * 128 + 32 * j:
                                       gl * 128 + 32 * (j + 1)],
                            rhs=rhs_ap(0, 128, h, w),
                            start=False, stop=False,
                            tile_position=(0, 32 * j),
                        )
                        nc.tensor.matmul(
                            ps[32 * j:32 * (j + 1), r * B:(r + 1) * B],
                            lhsT=w2_sb[32 * j:32 * j + 16,
                                       g * 32:(g + 1) * 32],
                            rhs=rhs_ap(32 * j, 16, h + dy, w + dx),
                            start=False, stop=False,
                            tile_position=(32 * j, 32 * j),
                        )
                # close the group across all 128 partitions (adds zeros)
                nc.tensor.matmul(
                    ps[:, 0:B],
                    lhsT=zb[:, 0:128],
                    rhs=zb[:, 128:128 + B],
                    start=False, stop=True,
                    tile_position=(0, 0),
                )
                bias_bc = (
                    bias_sb[:, g0:g0 + nb]
                    .unsqueeze(-1)
                    .broadcast_to([128, nb, B])
                )
                nc.vector.tensor_add(
                    out_sb[:, g0 * B:(g0 + nb) * B],
                    ps[:, :nb * B],
                    bias_bc,
                )
                nc.sync.dma_start(
                    out_d[:, g0 * B:(g0 + nb) * B],
                    out_sb[:, g0 * B:(g0 + nb) * B],
                )
    nc.compile()
    return nc


def make_in_maps(x, weight, bias):
    x = np.ascontiguousarray(np.asarray(x, np.float32))
    weight = np.asarray(weight, np.float32)
    bias = np.asarray(bias, np.float32)
    # pad rows/cols so shifted windows stay in-bounds (junk never read)
    xpad = np.pad(x, ((0, 0), (0, 0), (0, 2), (0, 2)))
    in_maps = []
    for ci in range(N_CORES):
        h0 = ROW_STARTS[ci]
        ws = weight[:, :, h0:h0 + ROWS, :, :]          # (O, C, 8, WO, 9)
        wsr = ws.reshape(O, C, 2, 4, WO, K * K)        # (o, c, hb, j, w, k)
        w1 = np.ascontiguousarray(
            wsr[..., :8].transpose(5, 1, 2, 4, 3, 0)   # (k, c, hb, w, j, o)
        ).reshape(128, G * 128)
        w2 = np.zeros((4, 32, G * 32), np.float32)
        w2[:, :16, :] = (
            wsr[..., 8].transpose(3, 1, 2, 4, 0)       # (j, c, hb, w, o)
            .reshape(4, 16, G * 32)
        )
        w2 = w2.reshape(128, G * 32)
        # im2col: xr[(k,c), (b, y, x)] = x[b, c, h0+y+ki, x+kj], k = ki*3+kj
        xr = np.empty((8, C, XREPW, YIN, B), np.float32)
        for k in range(8):
            ki, kj = divmod(k, K)
            xr[k] = xpad[:, :, h0 + ki:h0 + ki + YIN,
                         kj:kj + XREPW].transpose(1, 3, 2, 0)
        xr = np.ascontiguousarray(xr).reshape(128, B * YIN * XREPW)
        bv = np.ascontiguousarray(
            bias[0][:, h0:h0 + ROWS, :]
            .reshape(O, 2, 4, WO)                      # (o, hb, j, w)
            .transpose(2, 0, 1, 3)                     # (j, o, hb, w)
        ).reshape(128, G)
        if USE_BF16:
            import ml_dtypes
            w1 = w1.astype(ml_dtypes.bfloat16)
            w2 = w2.astype(ml_dtypes.bfloat16)
            xr = xr.astype(ml_dtypes.bfloat16)
        in_maps.append({"w1": w1, "w2": w2, "xr": xr, "bv": bv})
    return in_maps


def decode_outputs(results):
    out_full = np.empty((B, O, HO, WO), np.float32)
    for ci in range(N_CORES):
        h0 = ROW_STARTS[ci]
        o_dev = np.asarray(results[ci]["out"])          # (128, G*B)
        dec = (
            o_dev.reshape(4, O, 2, WO, B)               # (j, o, hb, w, b)
            .transpose(4, 1, 2, 0, 3)                   # (b, o, hb, j, w)
            .reshape(B, O, ROWS, WO)
        )
        out_full[:, :, h0:h0 + ROWS, :] = dec
    return out_full


def kernel(x=None, weight=None, bias=None):
    global _PROGRAM, _PROGRAM_KEY, LAST_RESULT
    if _PROGRAM is None or _PROGRAM_KEY != USE_BF16:
        _PROGRAM = build_program()
        _PROGRAM_KEY = USE_BF16
    in_maps = make_in_maps(x, weight, bias)
    res = run_bass_kernel_spmd(
        _PROGRAM, in_maps, core_ids=list(range(N_CORES)),
        trace=TRACE, **TRACE_KWARGS,
    )
    LAST_RESULT = res
    return decode_outputs(res.results)



# revision 2
# speedup vs baseline: 1.8607x; 1.8607x over previous
"""LocallyConnected2d kernel for 8 Trainium2 NeuronCores.

Problem:  out[b,o,h,w] = sum_{c,k} x[b, c, h+ki, w+kj] * W[o, c, h, w, k] + bias[o,h,w]
  B=8, C=16, H=W=64, O=32, K=3, HO=WO=62.

Sharding: each core owns 8 output rows (H_out axis), full O and batch.
  Weight slice (~9.1 MB/core) dominates HBM traffic -> memory roofline.

Compute mapping (per core): for each output location (h,w) the 144-term
reduction over (c, k) runs on the tensor engine as two accumulating fp32
matmuls; 4 locations (4 consecutive h rows, same w) run concurrently in the
four 32-column groups of the PE array via tile_position:
  chunk1: contraction (k<8, c) = 128 partitions, per-location weights
          stationary [128, o=32], patches moving [128, b=8].
  chunk2: contraction c = 16 partitions (k=8). Column group j reads its
          patches from xrep's k=2j block at a compensating offset
          (shift identity: x[h+2,w+2] = block(s)[h+dy, w+dx], (dy,dx)=(2,2)-s)
          so no separate narrow x tensor is needed; its weights sit in row
          block 32j of a dense 128-partition tensor.
  64 rounds accumulate into one full PSUM bank (opened/closed by K=1
  zero-weight matmuls); one DVE tensor_tensor per bank adds bias (broadcast
  AP along b) while evacuating.

The im2col tensor xrep[(k,c), (b, y<10, x<64)] is built on the HOST so it
lands in one contiguous full-width DMA.  Weight stream rides the SP HWDGE
ring; everything else rides the ACT ring so the transfers overlap.
"""

import numpy as np

import concourse.bacc as bacc
import concourse.mybir as mybir
from concourse import tile
from concourse.bass_utils import run_bass_kernel_spmd

F32 = mybir.dt.float32
BF16 = mybir.dt.bfloat16
USE_BF16 = True           # compute/weight dtype for w1/w2/xrep

N_CORES = 8
B, C, H, W = 8, 16, 64, 64
O, K = 32, 3
HO = WO = 62
ROWS = 8                # output rows per core
YIN = ROWS + K - 1      # input rows needed per core (10)
G = 2 * WO              # rounds per core (hb in {0,1} x w in [0,62)) = 124
ROW_STARTS = [0, 8, 16, 24, 32, 40, 48, 54]
W1_SIZES = [17, 17, 17, 17, 17, 12, 16, 11]       # rounds per chunk
W1_STARTS = list(np.cumsum([0] + W1_SIZES[:-1]))
XREPW = 64              # padded x width in xrep (62+2 used)
BANK_SIZES = [31, 31, 31, 31]       # rounds per PSUM bank
BANK_STARTS = list(np.cumsum([0] + BANK_SIZES[:-1]))
RPB = 31                # w2 transfer quarter (bank-independent)
# chunk2 offset (dy, dx) = (2,2) - shift(k=2j) per column group j
C2_DELTA = [(2, 2), (2, 0), (1, 1), (0, 2)]

TRACE = False
TRACE_KWARGS = {}
LAST_RESULT = None
_PROGRAM = None
_PROGRAM_KEY = None


def build_program():
    DT = BF16 if USE_BF16 else F32
    nc = bacc.Bacc(
        "TRN2", target_bir_lowering=False, debug=False, num_devices=N_CORES
    )
    w1_d = nc.dram_tensor("w1", [128, G * 128], DT, kind="ExternalInput")
    w2_d = nc.dram_tensor("w2", [128, G * 32], DT, kind="ExternalInput")
    xr_d = nc.dram_tensor("xr", [128, B * YIN * XREPW], DT,
                          kind="ExternalInput")
    bv_d = nc.dram_tensor("bv", [128, G], F32, kind="ExternalInput")
    out_d = nc.dram_tensor("out", [128, G * B], F32, kind="ExternalOutput")

    with tile.TileContext(nc) as tc:
        with tc.tile_pool(name="const", bufs=1) as cpool, \
             tc.tile_pool(name="w1p", bufs=8) as w1p, \
             tc.tile_pool(name="psum", bufs=1, space="PSUM") as ppool:
            # xrep free layout is (x, y, b): x-range splits are
            # contiguous, so the stream lands in 4 pieces and the PE can
            # start after the first ~320 KB; rhs reads are 8 contiguous
            # b elements.
            XP = XREPW // 2                       # x per piece (32)
            PCOLS = XP * YIN * B                  # cols per piece (1280)
            xps = [cpool.tile([128, PCOLS], DT, tag=f"xp{i}",
                              name=f"xp{i}")
                   for i in range(2)]
            w2_sb = cpool.tile([128, G * 32], DT)

            def load_xp(i):
                nc.scalar.dma_start(
                    xps[i][:], xr_d[:, i * PCOLS:(i + 1) * PCOLS])

            def load_w2(part):                    # bank-aligned quarters
                c0 = part * RPB * 32
                sz = min(RPB, G - part * RPB) * 32
                nc.scalar.dma_start(
                    w2_sb[:, c0:c0 + sz], w2_d[:, c0:c0 + sz])

            # order: everything round 0 touches first, then ascending w
            load_xp(0)
            load_w2(0)
            load_w2(1)
            load_xp(1)
            load_w2(2)
            load_w2(3)
            bias_sb = cpool.tile([128, G], F32)
            nc.scalar.dma_start(bias_sb[:], bv_d[:])
            out_sb = cpool.tile([128, G * B], F32)
            # bank open/close dummies use bf16 zeros: 1 cycle/row on the PE
            # instead of fp32's 4, and the values written are exact zeros
            zb = cpool.tile([1, 640], BF16)
            nc.vector.memset(zb[:], 0.0)

            def rhs_ap(p0, np_, y, x):
                i, xi = divmod(x, XP)
                c0 = (xi * YIN + y) * B
                return xps[i][p0:p0 + np_, c0:c0 + B]

            # weight chunks loaded lazily so each chunk's DMA overlaps the
            # previous chunk's matmuls
            w1_tiles = {}

            def get_w1(t):
                if t not in w1_tiles:
                    w1_sb = w1p.tile([128, max(W1_SIZES) * 128], DT,
                                     tag="w1")
                    c0, sz = W1_STARTS[t] * 128, W1_SIZES[t] * 128
                    # the last chunk rides the ACT ring so the two
                    # HWDGE queues finish together
                    eng = nc.scalar if t >= 6 else nc.sync
                    eng.dma_start(
                        w1_sb[:, :sz], w1_d[:, c0:c0 + sz]
                    )
                    w1_tiles[t] = w1_sb
                return w1_tiles[t]

            def w1_chunk_of(g):
                for t in range(len(W1_SIZES)):
                    if g < W1_STARTS[t] + W1_SIZES[t]:
                        return t, g - W1_STARTS[t]
                raise AssertionError(g)

            for bank in range(len(BANK_SIZES)):
                g0 = BANK_STARTS[bank]
                nb = BANK_SIZES[bank]
                ps = ppool.tile([128, 512], F32, tag=f"ps{bank}")
                # open the bank's accumulation group across all 128
                # partitions and clear the whole 2 KiB zero region
                nc.tensor.matmul(
                    ps[:, :],
                    lhsT=zb[:, 0:128],
                    rhs=zb[:, 128:640],
                    start=True, stop=False,
                    tile_position=(0, 0),
                )
                for r in range(nb):
                    g = g0 + r
                    hb, w = divmod(g, WO)
                    t, gl = w1_chunk_of(g)
                    w1_sb = get_w1(t)
                    for j in range(4):
                        h = hb * 4 + j
                        dy, dx = C2_DELTA[j]
                        nc.tensor.matmul(
                            ps[32 * j:32 * (j + 1), r * B:(r + 1) * B],
                            lhsT=w1_sb[:, gl * 128 + 32 * j:
                                       gl * 128 + 32 * (j + 1)],
                            rhs=rhs_ap(0, 128, h, w),
                            start=False, stop=False,
                            tile_position=(0, 32 * j),
                        )
                        nc.tensor.matmul(
                            ps[32 * j:32 * (j + 1), r * B:(r + 1) * B],
                            lhsT=w2_sb[32 * j:32 * j + 16,
                                       g * 32:(g + 1) * 32],
                            rhs=rhs_ap(32 * j, 16, h + dy, w + dx),
                            start=False, stop=False,
                            tile_position=(32 * j, 32 * j),
                        )
                # close the group across all 128 partitions (adds zeros)
                nc.tensor.matmul(
                    ps[:, 0:B],
                    lhsT=zb[:, 0:128],
                    rhs=zb[:, 128:128 + B],
                    start=False, stop=True,
                    tile_position=(0, 0),
                )
                bias_bc = (
                    bias_sb[:, g0:g0 + nb]
                    .unsqueeze(-1)
                    .broadcast_to([128, nb, B])
                )
                nc.vector.tensor_add(
                    out_sb[:, g0 * B:(g0 + nb) * B],
                    ps[:, :nb * B],
                    bias_bc,
                )
                nc.sync.dma_start(
                    out_d[:, g0 * B:(g0 + nb) * B],
                    out_sb[:, g0 * B:(g0 + nb) * B],
                )
    nc.compile()
    return nc


def make_in_maps(x, weight, bias):
    x = np.ascontiguousarray(np.asarray(x, np.float32))
    weight = np.asarray(weight, np.float32)
    bias = np.asarray(bias, np.float32)
    # pad rows/cols so shifted windows stay in-bounds (junk never read)
    xpad = np.pad(x, ((0, 0), (0, 0), (0, 2), (0, 2)))
    in_maps = []
    for ci in range(N_CORES):
        h0 = ROW_STARTS[ci]
        ws = weight[:, :, h0:h0 + ROWS, :, :]          # (O, C, 8, WO, 9)
        wsr = ws.reshape(O, C, 2, 4, WO, K * K)        # (o, c, hb, j, w, k)
        w1 = np.ascontiguousarray(
            wsr[..., :8].transpose(5, 1, 2, 4, 3, 0)   # (k, c, hb, w, j, o)
        ).reshape(128, G * 128)
        w2 = np.zeros((4, 32, G * 32), np.float32)
        w2[:, :16, :] = (
            wsr[..., 8].transpose(3, 1, 2, 4, 0)       # (j, c, hb, w, o)
            .reshape(4, 16, G * 32)
        )
        w2 = w2.reshape(128, G * 32)
        # im2col: xr[(k,c), (b, y, x)] = x[b, c, h0+y+ki, x+kj], k = ki*3+kj
        xr = np.empty((8, C, XREPW, YIN, B), np.float32)
        for k in range(8):
            ki, kj = divmod(k, K)
            xr[k] = xpad[:, :, h0 + ki:h0 + ki + YIN,
                         kj:kj + XREPW].transpose(1, 3, 2, 0)
        xr = np.ascontiguousarray(xr).reshape(128, B * YIN * XREPW)
        bv = np.ascontiguousarray(
            bias[0][:, h0:h0 + ROWS, :]
            .reshape(O, 2, 4, WO)                      # (o, hb, j, w)
            .transpose(2, 0, 1, 3)                     # (j, o, hb, w)
        ).reshape(128, G)
        if USE_BF16:
            import ml_dtypes
            w1 = w1.astype(ml_dtypes.bfloat16)
            w2 = w2.astype(ml_dtypes.bfloat16)
            xr = xr.astype(ml_dtypes.bfloat16)
        in_maps.append({"w1": w1, "w2": w2, "xr": xr, "bv": bv})
    return in_maps


def decode_outputs(results):
    out_full = np.empty((B, O, HO, WO), np.float32)
    for ci in range(N_CORES):
        h0 = ROW_STARTS[ci]
        o_dev = np.asarray(results[ci]["out"])          # (128, G*B)
        dec = (
            o_dev.reshape(4, O, 2, WO, B)               # (j, o, hb, w, b)
            .transpose(4, 1, 2, 0, 3)                   # (b, o, hb, j, w)
            .reshape(B, O, ROWS, WO)
        )
        out_full[:, :, h0:h0 + ROWS, :] = dec
    return out_full


def kernel(x=None, weight=None, bias=None):
    global _PROGRAM, _PROGRAM_KEY, LAST_RESULT
    if _PROGRAM is None or _PROGRAM_KEY != USE_BF16:
        _PROGRAM = build_program()
        _PROGRAM_KEY = USE_BF16
    in_maps = make_in_maps(x, weight, bias)
    res = run_bass_kernel_spmd(
        _PROGRAM, in_maps, core_ids=list(range(N_CORES)),
        trace=TRACE, **TRACE_KWARGS,
    )
    LAST_RESULT = res
    return decode_outputs(res.results)



# revision 3
# speedup vs baseline: 2.9730x; 1.5977x over previous
"""LocallyConnected2d kernel for 8 Trainium2 NeuronCores.

Problem:  out[b,o,h,w] = sum_{c,k} x[b, c, h+ki, w+kj] * W[o, c, h, w, k] + bias[o,h,w]
  B=8, C=16, H=W=64, O=32, K=3, HO=WO=62.

Sharding: each core owns 8 output rows (H_out axis), full O and batch.

Compute mapping (per core): for each output location (h,w) the 144-term
reduction over (c, k) runs on the tensor engine as two accumulating fp32
matmuls; 4 locations (4 consecutive h rows, same w) run concurrently in the
four 32-column groups of the PE array via tile_position:
  chunk1: contraction (k<8, c) = 128 partitions, per-location weights
          stationary [128, o=32], patches moving [128, b=8].
  chunk2: contraction c = 16 partitions (k=8). Column group j reads its
          patches from the im2col block k=2j at a compensating offset
          (shift identity), weights at partitions 32j of w2.

Cost-model-driven layout choices:
  - DMA cost = per-partition free bytes; three parallel queues exist
    (SP + ACT HWDGE, Pool).  Weights ride fp8(e3m4) (max rel err ~1.3e-2
    vs the 2e-2 gate), patches bf16, output fp32.
  - bias is appended (bf16) to the im2col tensor so it rides an existing
    transfer instead of paying the 500ns small-DMA floor.
  - each queue's final output DMA is placed so its dependency is ready
    before the queue drains, hiding the ~1.7us DGE init latency.
  - PSUM accumulation groups open/close on the real matmuls (start on
    chunk1, stop on chunk2) -- no zero-weight bank-clear matmuls.
  - a few zero matmuls at t=0 warm the PE p-state ramp.
"""

import numpy as np
import ml_dtypes

import concourse.bacc as bacc
import concourse.mybir as mybir
from concourse import tile
from concourse.bass_utils import run_bass_kernel_spmd

F32 = mybir.dt.float32
BF16 = mybir.dt.bfloat16
F8 = mybir.dt.float8e3

USE_F8 = True            # w1/w2 dtype: fp8 e3m4 (else bf16)
FUSED_OPEN = True        # open/close PSUM groups on real matmuls
N_WARM = 4               # PE warmup dummy matmuls

N_CORES = 8
B, C, H, W = 8, 16, 64, 64
O, K = 32, 3
HO = WO = 62
ROWS = 8                # output rows per core
YIN = ROWS + K - 1      # input rows needed per core (10)
G = 2 * WO              # rounds per core (hb in {0,1} x w in [0,62)) = 124
ROW_STARTS = [0, 8, 16, 24, 32, 40, 48, 54]
XREPW = 64              # padded x width in xrep (62+2 used)
XB = G                  # bias columns at the head of xr (124)
XRC = XB + B * YIN * XREPW   # 124 + 5120

# w1 split in rounds: first piece small for fast PE start
W1_SIZES = [6, 17, 17, 17, 17, 17, 17, 16]
W1_STARTS = list(np.cumsum([0] + W1_SIZES[:-1]))
# xrep pieces in x columns (piece 0 also carries the bias block)
XP_SIZES = [8, 14, 14, 14, 14]
XP_STARTS = list(np.cumsum([0] + XP_SIZES[:-1]))
W2_HALVES = [(0, 62), (62, 124)]          # rounds per w2 piece
BANK_SIZES = [31, 31, 31, 31]
BANK_STARTS = list(np.cumsum([0] + BANK_SIZES[:-1]))
# chunk2 offset (dy, dx) = (2,2) - shift(k=2j) per column group j
C2_DELTA = [(2, 2), (2, 0), (1, 1), (0, 2)]

# DMA plan: ordered input pieces per queue, then output pieces per queue.
# Queues: sp = SP HWDGE, act = ACT HWDGE, gps = Pool.
PLAN = {
    "sp": [("w1", 0), ("w1", 2), ("w1", 4), ("w1", 6), ("xp", 3)],
    "act": [("xp", 0), ("w1", 1), ("xp", 2), ("w1", 5), ("w1", 7)],
    "gps": [("w2", 0), ("xp", 1), ("w1", 3), ("w2", 1), ("xp", 4)],
}
# output DMAs (round ranges), appended to each queue after its inputs
OUT_PLAN = {
    "sp": [],
    "act": [(0, 62)],
    "gps": [(62, 93), (93, 124)],
}

TRACE = False
TRACE_KWARGS = {}
LAST_RESULT = None
_PROGRAM = None
_PROGRAM_KEY = None


def build_program():
    DT = F8 if USE_F8 else BF16
    nc = bacc.Bacc(
        "TRN2", target_bir_lowering=False, debug=False, num_devices=N_CORES
    )
    w1_d = nc.dram_tensor("w1", [128, G * 128], DT, kind="ExternalInput")
    w2_d = nc.dram_tensor("w2", [128, G * 32], DT, kind="ExternalInput")
    xr_d = nc.dram_tensor("xr", [128, XRC], BF16, kind="ExternalInput")
    out_d = nc.dram_tensor("out", [128, G * B], F32, kind="ExternalOutput")

    with tile.TileContext(nc) as tc:
        with tc.tile_pool(name="const", bufs=1) as cpool, \
             tc.tile_pool(name="psum", bufs=1, space="PSUM") as ppool:
            w1_sb = cpool.tile([128, G * 128], DT)
            w2_sb = cpool.tile([128, G * 32], DT)
            xr_sb = cpool.tile([128, XRC], BF16)
            out_sb = cpool.tile([128, G * B], F32)
            zb = cpool.tile([1, 640], BF16)
            nc.vector.memset(zb[:], 0.0)

            engines = {"sp": nc.sync, "act": nc.scalar, "gps": nc.gpsimd}

            def emit_piece(eng, kind, idx):
                if kind == "w1":
                    c0 = W1_STARTS[idx] * 128
                    sz = W1_SIZES[idx] * 128
                    eng.dma_start(w1_sb[:, c0:c0 + sz], w1_d[:, c0:c0 + sz])
                elif kind == "w2":
                    g0, g1 = W2_HALVES[idx]
                    eng.dma_start(w2_sb[:, g0 * 32:g1 * 32],
                                  w2_d[:, g0 * 32:g1 * 32])
                elif kind == "xp":
                    c0 = XB + XP_STARTS[idx] * YIN * B
                    sz = XP_SIZES[idx] * YIN * B
                    if idx == 0:
                        c0 = 0
                        sz += XB
                    eng.dma_start(xr_sb[:, c0:c0 + sz], xr_d[:, c0:c0 + sz])
                else:
                    raise AssertionError(kind)

            for q, ops in PLAN.items():
                for kind, idx in ops:
                    emit_piece(engines[q], kind, idx)

            # PE warmup: keep the p-state ramp counting from ~t=0
            ps_w = ppool.tile([128, 512], F32, tag="psw")
            for _ in range(N_WARM):
                nc.tensor.matmul(
                    ps_w[:, :],
                    lhsT=zb[:, 0:128],
                    rhs=zb[:, 128:640],
                    start=True, stop=True,
                    tile_position=(0, 0),
                )

            def rhs_ap(p0, np_, y, x):
                c0 = XB + (x * YIN + y) * B
                return xr_sb[p0:p0 + np_, c0:c0 + B]

            for bank in range(len(BANK_SIZES)):
                g0 = BANK_STARTS[bank]
                nb = BANK_SIZES[bank]
                ps = ppool.tile([128, 512], F32, tag=f"ps{bank}")
                if not FUSED_OPEN:
                    nc.tensor.matmul(
                        ps[:, :], lhsT=zb[:, 0:128], rhs=zb[:, 128:640],
                        start=True, stop=False, tile_position=(0, 0),
                    )
                for r in range(nb):
                    g = g0 + r
                    hb, w = divmod(g, WO)
                    for j in range(4):
                        h = hb * 4 + j
                        dy, dx = C2_DELTA[j]
                        nc.tensor.matmul(
                            ps[32 * j:32 * (j + 1), r * B:(r + 1) * B],
                            lhsT=w1_sb[:, g * 128 + 32 * j:
                                       g * 128 + 32 * (j + 1)],
                            rhs=rhs_ap(0, 128, h, w),
                            start=FUSED_OPEN, stop=False,
                            tile_position=(0, 32 * j),
                        )
                        nc.tensor.matmul(
                            ps[32 * j:32 * (j + 1), r * B:(r + 1) * B],
                            lhsT=w2_sb[32 * j:32 * j + 16,
                                       g * 32:(g + 1) * 32],
                            rhs=rhs_ap(32 * j, 16, h + dy, w + dx),
                            start=False, stop=FUSED_OPEN,
                            tile_position=(32 * j, 32 * j),
                        )
                if not FUSED_OPEN:
                    nc.tensor.matmul(
                        ps[:, 0:B], lhsT=zb[:, 0:128], rhs=zb[:, 128:128 + B],
                        start=False, stop=True, tile_position=(0, 0),
                    )
                bias_bc = (
                    xr_sb[:, g0:g0 + nb]
                    .unsqueeze(-1)
                    .broadcast_to([128, nb, B])
                )
                nc.vector.tensor_add(
                    out_sb[:, g0 * B:(g0 + nb) * B],
                    ps[:, :nb * B],
                    bias_bc,
                )

            for q, ranges in OUT_PLAN.items():
                for ga, gb in ranges:
                    engines[q].dma_start(
                        out_d[:, ga * B:gb * B], out_sb[:, ga * B:gb * B]
                    )
    nc.compile()
    return nc


def make_in_maps(x, weight, bias):
    x = np.ascontiguousarray(np.asarray(x, np.float32))
    weight = np.asarray(weight, np.float32)
    bias = np.asarray(bias, np.float32)
    WDT = ml_dtypes.float8_e3m4 if USE_F8 else ml_dtypes.bfloat16
    # pad rows/cols so shifted windows stay in-bounds (junk never read)
    xpad = np.pad(x, ((0, 0), (0, 0), (0, 2), (0, 2)))
    in_maps = []
    for ci in range(N_CORES):
        h0 = ROW_STARTS[ci]
        ws = weight[:, :, h0:h0 + ROWS, :, :]          # (O, C, 8, WO, 9)
        wsr = ws.reshape(O, C, 2, 4, WO, K * K)        # (o, c, hb, j, w, k)
        w1 = np.ascontiguousarray(
            wsr[..., :8].transpose(5, 1, 2, 4, 3, 0)   # (k, c, hb, w, j, o)
        ).reshape(128, G * 128).astype(WDT)
        w2 = np.zeros((4, 32, G * 32), np.float32)
        w2[:, :16, :] = (
            wsr[..., 8].transpose(3, 1, 2, 4, 0)       # (j, c, hb, w, o)
            .reshape(4, 16, G * 32)
        )
        w2 = w2.reshape(128, G * 32).astype(WDT)
        # im2col: xr[(k,c), (x, y, b)] = x[b, c, h0+y+ki, x+kj], k = ki*3+kj
        xr = np.empty((8, C, XREPW, YIN, B), np.float32)
        for k in range(8):
            ki, kj = divmod(k, K)
            xr[k] = xpad[:, :, h0 + ki:h0 + ki + YIN,
                         kj:kj + XREPW].transpose(1, 3, 2, 0)
        xr = np.ascontiguousarray(xr).reshape(128, B * YIN * XREPW)
        bv = np.ascontiguousarray(
            bias[0][:, h0:h0 + ROWS, :]
            .reshape(O, 2, 4, WO)                      # (o, hb, j, w)
            .transpose(2, 0, 1, 3)                     # (j, o, hb, w)
        ).reshape(128, G)
        xrfull = np.concatenate([bv, xr], axis=1).astype(ml_dtypes.bfloat16)
        in_maps.append({"w1": w1, "w2": w2, "xr": xrfull})
    return in_maps


def decode_outputs(results):
    out_full = np.empty((B, O, HO, WO), np.float32)
    for ci in range(N_CORES):
        h0 = ROW_STARTS[ci]
        o_dev = np.asarray(results[ci]["out"])          # (128, G*B)
        dec = (
            o_dev.reshape(4, O, 2, WO, B)               # (j, o, hb, w, b)
            .transpose(4, 1, 2, 0, 3)                   # (b, o, hb, j, w)
            .reshape(B, O, ROWS, WO)
        )
        out_full[:, :, h0:h0 + ROWS, :] = dec
    return out_full


def _program_key():
    return (USE_F8, FUSED_OPEN, N_WARM,
            tuple(W1_SIZES), tuple(XP_SIZES),
            tuple(sorted((k, tuple(v)) for k, v in PLAN.items())),
            tuple(sorted((k, tuple(v)) for k, v in OUT_PLAN.items())))


def kernel(x=None, weight=None, bias=None):
    global _PROGRAM, _PROGRAM_KEY, LAST_RESULT
    if _PROGRAM is None or _PROGRAM_KEY != _program_key():
        _PROGRAM = build_program()
        _PROGRAM_KEY = _program_key()
    in_maps = make_in_maps(x, weight, bias)
    res = run_bass_kernel_spmd(
        _PROGRAM, in_maps, core_ids=list(range(N_CORES)),
        trace=TRACE, **TRACE_KWARGS,
    )
    LAST_RESULT = res
    return decode_outputs(res.results)


# revision 45
# speedup vs baseline: 3.1298x; 1.0527x over previous
"""LocallyConnected2d kernel for 8 Trainium2 NeuronCores.

Problem:  out[b,o,h,w] = sum_{c,k} x[b, c, h+ki, w+kj] * W[o, c, h, w, k] + bias[o,h,w]
  B=8, C=16, H=W=64, O=32, K=3, HO=WO=62.

Sharding: each core owns 8 output rows (H_out axis), full O and batch.

Compute mapping (per core): for each output location (h,w) the 144-term
reduction over (c, k) runs on the tensor engine as two accumulating fp32
matmuls; 4 locations (4 consecutive h rows, same w) run concurrently in the
four 32-column groups of the PE array via tile_position:
  chunk1: contraction (k<8, c) = 128 partitions, per-location weights
          stationary [128, o=32], patches moving [128, b=8].
  chunk2: contraction c = 16 partitions (k=8). Column group j reads its
          patches from the im2col block k=2j at a compensating offset
          (shift identity), weights at partitions 32j of w2.

Cost-model-driven layout choices:
  - DMA cost = per-partition free bytes; three parallel queues exist
    (SP + ACT HWDGE, Pool).  Weights ride fp8(e3m4) (max rel err ~1.3e-2
    vs the 2e-2 gate), patches bf16, output fp32.
  - bias is appended (bf16) to the im2col tensor so it rides an existing
    transfer instead of paying the 500ns small-DMA floor.
  - each queue's final output DMA is placed so its dependency is ready
    before the queue drains, hiding the ~1.7us DGE init latency.
  - PSUM accumulation groups open/close on the real matmuls (start on
    chunk1, stop on chunk2) -- no zero-weight bank-clear matmuls.
  - a few zero matmuls at t=0 warm the PE p-state ramp.
"""

import numpy as np
import ml_dtypes

import concourse.bacc as bacc
import concourse.mybir as mybir
from concourse import tile
from concourse.bass_utils import run_bass_kernel_spmd

F32 = mybir.dt.float32
BF16 = mybir.dt.bfloat16
F8 = mybir.dt.float8e3

USE_F8 = True            # w1/w2 dtype: fp8 e3m4 (else bf16)
FUSED_OPEN = True        # open/close PSUM groups on real matmuls
N_WARM = 4               # PE warmup dummy matmuls
WARM_COLS = 512          # moving width of each warmup matmul
WARM_LAST = 190          # moving width of the final warmup matmul

N_CORES = 8
B, C, H, W = 8, 16, 64, 64
O, K = 32, 3
HO = WO = 62
ROWS = 8                # output rows per core
YIN = ROWS + K - 1      # input rows needed per core (10)
G = 2 * WO              # rounds per core (hb in {0,1} x w in [0,62)) = 124
ROW_STARTS = [0, 8, 16, 24, 32, 40, 48, 54]
XREPW = 64              # padded x width in xrep (62+2 used)
XB = G                  # bias columns at the head of xr (124)
XRC = XB + B * YIN * XREPW   # 124 + 5120

# w1 split in rounds: first piece small for fast PE start
W1_SIZES = [11, 17, 17, 17, 17, 17, 17, 11]
W1_STARTS = list(np.cumsum([0] + W1_SIZES[:-1]))
# xrep pieces in x columns (piece 0 also carries the bias block)
XP_SIZES = [8, 14, 14, 14, 14]
XP_STARTS = list(np.cumsum([0] + XP_SIZES[:-1]))
W2_HALVES = [(0, 62), (62, 124)]          # rounds per w2 piece
BANK_SIZES = [33, 33, 34, 24]
BANK_STARTS = list(np.cumsum([0] + BANK_SIZES[:-1]))
# chunk2 offset (dy, dx) = (2,2) - shift(k=2j) per column group j
C2_DELTA = [(2, 2), (2, 0), (1, 1), (0, 2)]
# per-bank evacuation splits (bank -> slice sizes in rounds)
EVAC_SPLITS = {}

# DMA plan: ordered input pieces per queue, then output pieces per queue.
# Queues: sp = SP HWDGE, act = ACT HWDGE, gps = Pool.
PLAN = {
    "sp": [("w1", 0), ("w1", 2), ("w1", 4), ("w1", 6), ("xp", 3)],
    "act": [("xp", 0), ("w1", 1), ("xp", 2), ("w1", 5), ("w1", 7)],
    "gps": [("w2", 0), ("xp", 1), ("w1", 3), ("w2", 1), ("xp", 4)],
}
# output DMAs (round ranges), appended to each queue after its inputs
OUT_PLAN = {
    "sp": [(0, 66)],
    "act": [(100, 124)],
    "gps": [(66, 100)],
}

TRACE = False
TRACE_KWARGS = {}
LAST_RESULT = None
_PROGRAM = None
_PROGRAM_KEY = None


def _evac_slices(bank, nb):
    sizes = EVAC_SPLITS.get(bank, [nb])
    assert sum(sizes) == nb
    out, s0 = [], 0
    for sn in sizes:
        out.append((s0, sn))
        s0 += sn
    return out


def build_program():
    DT = F8 if USE_F8 else BF16
    nc = bacc.Bacc(
        "TRN2", target_bir_lowering=False, debug=False, num_devices=N_CORES
    )
    w1_d = nc.dram_tensor("w1", [128, G * 128], DT, kind="ExternalInput")
    w2_d = nc.dram_tensor("w2", [128, G * 32], DT, kind="ExternalInput")
    xr_d = nc.dram_tensor("xr", [128, XRC], BF16, kind="ExternalInput")
    out_d = nc.dram_tensor("out", [128, G * B], F32, kind="ExternalOutput")

    with tile.TileContext(nc) as tc:
        with tc.tile_pool(name="const", bufs=1) as cpool, \
             tc.tile_pool(name="psum", bufs=1, space="PSUM") as ppool:
            w1_sb = cpool.tile([128, G * 128], DT)
            w2_sb = cpool.tile([128, G * 32], DT)
            xr_sb = cpool.tile([128, XRC], BF16)
            out_sb = cpool.tile([128, G * B], F32)
            zb = None
            if N_WARM > 0 or not FUSED_OPEN:
                zb = cpool.tile([1, 128 + max(WARM_COLS, 512 * int(not FUSED_OPEN))], BF16)
                nc.vector.memset(zb[:], 0.0)

            engines = {"sp": nc.sync, "act": nc.scalar, "gps": nc.gpsimd}

            def emit_piece(eng, kind, idx):
                if kind == "w1":
                    c0 = W1_STARTS[idx] * 128
                    sz = W1_SIZES[idx] * 128
                    eng.dma_start(w1_sb[:, c0:c0 + sz], w1_d[:, c0:c0 + sz])
                elif kind == "w2":
                    g0, g1 = W2_HALVES[idx]
                    eng.dma_start(w2_sb[:, g0 * 32:g1 * 32],
                                  w2_d[:, g0 * 32:g1 * 32])
                elif kind == "xp":
                    c0 = XB + XP_STARTS[idx] * YIN * B
                    sz = XP_SIZES[idx] * YIN * B
                    if idx == 0:
                        c0 = 0
                        sz += XB
                    eng.dma_start(xr_sb[:, c0:c0 + sz], xr_d[:, c0:c0 + sz])
                else:
                    raise AssertionError(kind)

            for q, ops in PLAN.items():
                for kind, idx in ops:
                    emit_piece(engines[q], kind, idx)

            # PE warmup: keep the p-state ramp counting from ~t=0
            if N_WARM > 0:
                ps_w = ppool.tile([128, 512], F32, tag="psw")
                for i in range(N_WARM):
                    wc = WARM_LAST if i == N_WARM - 1 else WARM_COLS
                    nc.tensor.matmul(
                        ps_w[:, :wc],
                        lhsT=zb[:, 0:128],
                        rhs=zb[:, 128:128 + wc],
                        start=True, stop=True,
                        tile_position=(0, 0),
                    )

            def rhs_ap(p0, np_, y, x):
                c0 = XB + (x * YIN + y) * B
                return xr_sb[p0:p0 + np_, c0:c0 + B]

            for bank in range(len(BANK_SIZES)):
                g0 = BANK_STARTS[bank]
                nb = BANK_SIZES[bank]
                ps = ppool.tile([128, 512], F32, tag=f"ps{bank}")
                if not FUSED_OPEN:
                    nc.tensor.matmul(
                        ps[:, :], lhsT=zb[:, 0:128], rhs=zb[:, 128:640],
                        start=True, stop=False, tile_position=(0, 0),
                    )
                for r in range(nb):
                    g = g0 + r
                    hb, w = divmod(g, WO)
                    for j in range(4):
                        h = hb * 4 + j
                        dy, dx = C2_DELTA[j]
                        nc.tensor.matmul(
                            ps[32 * j:32 * (j + 1), r * B:(r + 1) * B],
                            lhsT=w1_sb[:, g * 128 + 32 * j:
                                       g * 128 + 32 * (j + 1)],
                            rhs=rhs_ap(0, 128, h, w),
                            start=FUSED_OPEN, stop=False,
                            tile_position=(0, 32 * j),
                        )
                        nc.tensor.matmul(
                            ps[32 * j:32 * (j + 1), r * B:(r + 1) * B],
                            lhsT=w2_sb[32 * j:32 * j + 16,
                                       g * 32:(g + 1) * 32],
                            rhs=rhs_ap(32 * j, 16, h + dy, w + dx),
                            start=False, stop=FUSED_OPEN,
                            tile_position=(32 * j, 32 * j),
                        )
                if not FUSED_OPEN:
                    nc.tensor.matmul(
                        ps[:, 0:B], lhsT=zb[:, 0:128], rhs=zb[:, 128:128 + B],
                        start=False, stop=True, tile_position=(0, 0),
                    )
                for s0, sn in _evac_slices(bank, nb):
                    bias_bc = (
                        xr_sb[:, g0 + s0:g0 + s0 + sn]
                        .unsqueeze(-1)
                        .broadcast_to([128, sn, B])
                    )
                    nc.vector.tensor_add(
                        out_sb[:, (g0 + s0) * B:(g0 + s0 + sn) * B],
                        ps[:, s0 * B:(s0 + sn) * B],
                        bias_bc,
                    )

            for q, ranges in OUT_PLAN.items():
                for ga, gb in ranges:
                    engines[q].dma_start(
                        out_d[:, ga * B:gb * B], out_sb[:, ga * B:gb * B]
                    )
    nc.compile()
    return nc


def make_in_maps(x, weight, bias):
    x = np.ascontiguousarray(np.asarray(x, np.float32))
    weight = np.asarray(weight, np.float32)
    bias = np.asarray(bias, np.float32)
    WDT = ml_dtypes.float8_e3m4 if USE_F8 else ml_dtypes.bfloat16
    # pad rows/cols so shifted windows stay in-bounds (junk never read)
    xpad = np.pad(x, ((0, 0), (0, 0), (0, 2), (0, 2)))
    in_maps = []
    for ci in range(N_CORES):
        h0 = ROW_STARTS[ci]
        ws = weight[:, :, h0:h0 + ROWS, :, :]          # (O, C, 8, WO, 9)
        wsr = ws.reshape(O, C, 2, 4, WO, K * K)        # (o, c, hb, j, w, k)
        w1 = np.ascontiguousarray(
            wsr[..., :8].transpose(5, 1, 2, 4, 3, 0)   # (k, c, hb, w, j, o)
        ).reshape(128, G * 128).astype(WDT)
        w2 = np.zeros((4, 32, G * 32), np.float32)
        w2[:, :16, :] = (
            wsr[..., 8].transpose(3, 1, 2, 4, 0)       # (j, c, hb, w, o)
            .reshape(4, 16, G * 32)
        )
        w2 = w2.reshape(128, G * 32).astype(WDT)
        # im2col: xr[(k,c), (x, y, b)] = x[b, c, h0+y+ki, x+kj], k = ki*3+kj
        xr = np.empty((8, C, XREPW, YIN, B), np.float32)
        for k in range(8):
            ki, kj = divmod(k, K)
            xr[k] = xpad[:, :, h0 + ki:h0 + ki + YIN,
                         kj:kj + XREPW].transpose(1, 3, 2, 0)
        xr = np.ascontiguousarray(xr).reshape(128, B * YIN * XREPW)
        bv = np.ascontiguousarray(
            bias[0][:, h0:h0 + ROWS, :]
            .reshape(O, 2, 4, WO)                      # (o, hb, j, w)
            .transpose(2, 0, 1, 3)                     # (j, o, hb, w)
        ).reshape(128, G)
        xrfull = np.concatenate([bv, xr], axis=1).astype(ml_dtypes.bfloat16)
        in_maps.append({"w1": w1, "w2": w2, "xr": xrfull})
    return in_maps


def decode_outputs(results):
    out_full = np.empty((B, O, HO, WO), np.float32)
    for ci in range(N_CORES):
        h0 = ROW_STARTS[ci]
        o_dev = np.asarray(results[ci]["out"])          # (128, G*B)
        dec = (
            o_dev.reshape(4, O, 2, WO, B)               # (j, o, hb, w, b)
            .transpose(4, 1, 2, 0, 3)                   # (b, o, hb, j, w)
            .reshape(B, O, ROWS, WO)
        )
        out_full[:, :, h0:h0 + ROWS, :] = dec
    return out_full


def _program_key():
    return (USE_F8, FUSED_OPEN, N_WARM, WARM_COLS, WARM_LAST,
            tuple(W1_SIZES), tuple(XP_SIZES),
            tuple(sorted((k, tuple(v)) for k, v in PLAN.items())),
            tuple(sorted((k, tuple(v)) for k, v in OUT_PLAN.items())))


def kernel(x=None, weight=None, bias=None):
    global _PROGRAM, _PROGRAM_KEY, LAST_RESULT
    if _PROGRAM is None or _PROGRAM_KEY != _program_key():
        _PROGRAM = build_program()
        _PROGRAM_KEY = _program_key()
    in_maps = make_in_maps(x, weight, bias)
    res = run_bass_kernel_spmd(
        _PROGRAM, in_maps, core_ids=list(range(N_CORES)),
        trace=TRACE, **TRACE_KWARGS,
    )
    LAST_RESULT = res
    return decode_outputs(res.results)
